# revision 13
# baseline (speedup 1.0000x reference)
"""Multi-head graph attention kernel for Trainium2, SPMD over 8 NeuronCores.

Sharding: core c owns batch b=c//4 and destination-row slice
i in [512*(c%4), 512*(c%4+1)), all 8 heads.  Each core computes complete
softmax rows (j ranges over all 2048 sources), so there are no cross-core
collectives; the host concatenates the per-core [512, 256] output slabs.

Score pipeline (per core, layout [j=partition, i=free]):
  exp(lrelu(e_src_i + e_dst_j)) = max(A_j*B_i, C_j*D_i)
     A=exp(e_dst), B=exp(e_src), C=exp(alpha*e_dst), D=exp(alpha*e_src)
  Factor out A_j (positive):  max(A*B, C*D) = A * max(B, R*D),
     R_j = C_j/A_j = exp((alpha-1)*e_dst_j)
  P2_ji = exp(beta*ln(prior_ji + eps)) * adj_ji    (mask as multiply)
  Two fused DVE ops per (h, jt) tile:
     v  = (Db * R_j) max Bb          [scalar_tensor_tensor]
     s  = (v * A_j) * P2             [scalar_tensor_tensor]
  hT[f,i] = sum_j xp[j,f] * s_ji   (PE, ones-column gives Z_i = sum_j s_ji)
  out[i,:] = (hT[:,i]/Z) @ W_out.T  (PE, hT layout feeds lhsT directly)

Engine assignment: DVE does the two score ops (+casts, xp copies, recip);
scalar does all activations (batched by table set: one Exp group, one Ln
group, one Exp group) and PSUM->SBUF copies; GpSimd does the adj-mask
multiply and the Z-normalize multiply; broadcasts B/D rows across
partitions via stride-0 DMA instead of PE matmuls.
"""

import math
import sys
from contextlib import ExitStack

sys.path.insert(0, "/opt/trn_rl_repo")

import numpy as np

import concourse.bass as bass
import concourse.tile as tile
from concourse import bacc, mybir
from concourse.bass_utils import run_bass_kernel_spmd

B, N, D, H = 2, 2048, 256, 8
DH = D // H          # 32
NC = 8
ISL = N // 4         # 512 destination rows per core
NJ = N // 128        # 16 j-tiles
EPS = 1e-6
ALPHA = 0.2
VSTASH_H = 2         # heads whose v-tiles are precomputed into a stash

F32 = mybir.dt.float32
BF16 = mybir.dt.bfloat16

AF = mybir.ActivationFunctionType
OP = mybir.AluOpType

_cache = {}
last_run_info = {}


def _build(beta: float, dbg: bool = False):
    nc = bacc.Bacc(
        "TRN2",
        target_bir_lowering=False,
        debug=False,
        enable_asserts=False,
        num_devices=NC,
    )

    def inp(name, shape, dt):
        return nc.dram_tensor(name, shape, dt, kind="ExternalInput").ap()

    xT_d = inp("xT", [D, N], F32)          # x[b].T
    xTs_d = inp("xTs", [D, ISL], F32)      # x[b, i_slice].T
    prT_d = inp("prT", [N, ISL], F32)      # prior[b, i_slice, :].T
    adT_d = inp("adT", [N, ISL], BF16)     # adj[i_slice, :].T as 0.0/1.0
    Wall_d = inp("Wall", [D, D], F32)      # W head-major columns
    WoT_d = inp("WoT", [D, D], F32)        # W_out.T
    wsT_d = inp("wsT", [D, H], F32)        # (W@a_src per head).T
    wdT_d = inp("wdT", [D, H], F32)        # (W@a_dst per head).T
    out_d = nc.dram_tensor("out", [ISL, D], F32, kind="ExternalOutput").ap()
    # DRAM bounce buffer for partition-broadcasts (SBUF APs cannot have
    # stride-0 partition dims, DRAM APs can)
    bd_scr = nc.dram_tensor("bdscr", [2 * H, ISL], BF16, kind="Internal").ap()
    z_scr = nc.dram_tensor("zscr", [H, ISL], F32, kind="Internal").ap()

    with tile.TileContext(nc) as tc, ExitStack() as ctx:
        pp = ctx.enter_context(tc.tile_pool(name="persist", bufs=1))
        wk = ctx.enter_context(tc.tile_pool(name="work", bufs=3))

        # ---- resident inputs
        xT = [pp.tile([128, N], F32, tag=f"xT{k}", name=f"xT{k}") for k in range(2)]
        xTs = [pp.tile([128, ISL], F32, tag=f"xTs{k}", name=f"xTs{k}") for k in range(2)]
        Wall = [pp.tile([128, D], F32, tag=f"Wall{k}", name=f"Wall{k}") for k in range(2)]
        WoT = [pp.tile([128, D], F32, tag=f"WoT{k}", name=f"WoT{k}") for k in range(2)]
        wsT = [pp.tile([128, H], F32, tag=f"wsT{k}", name=f"wsT{k}") for k in range(2)]
        wdT = [pp.tile([128, H], F32, tag=f"wdT{k}", name=f"wdT{k}") for k in range(2)]
        for k in range(2):
            r = slice(k * 128, (k + 1) * 128)
            nc.sync.dma_start(xT[k][:], xT_d[r, :])
            nc.sync.dma_start(xTs[k][:], xTs_d[r, :])
            nc.sync.dma_start(Wall[k][:], Wall_d[r, :])
            nc.sync.dma_start(WoT[k][:], WoT_d[r, :])
            nc.sync.dma_start(wsT[k][:], wsT_d[r, :])
            nc.sync.dma_start(wdT[k][:], wdT_d[r, :])

        prT_sb = pp.tile([128, NJ * ISL], F32, tag="prT", name="prT")
        adTf = pp.tile([128, NJ * ISL], BF16, tag="adTf", name="adTf")
        for jt in range(NJ):
            r = slice(jt * 128, (jt + 1) * 128)
            ci = slice(jt * ISL, (jt + 1) * ISL)
            nc.sync.dma_start(prT_sb[:, ci], prT_d[r, :])
            nc.sync.dma_start(adTf[:, ci], adT_d[r, :])

        # bf16 copies of x.T / W for the projection matmul (fp32 matmuls
        # run as HI/LO pairs on the PE - 2x the instructions and cycles)
        xbf = [pp.tile([128, N], BF16, tag=f"xbf{k}", name=f"xbf{k}")
               for k in range(2)]
        Wbf = [pp.tile([128, D], BF16, tag=f"Wbf{k}", name=f"Wbf{k}")
               for k in range(2)]
        for k in range(2):
            nc.vector.tensor_copy(xbf[k][:], xT[k][:])
            nc.vector.tensor_copy(Wbf[k][:], Wall[k][:])

        # ---- persistent intermediates
        xp_aug = pp.tile([128, NJ * H * 33], BF16, tag="xpaug", name="xpaug")
        nc.vector.memset(xp_aug[:], 1.0)  # ones col per 33-block survives
        epsb = pp.tile([128, 1], F32, tag="epsb", name="epsb")
        nc.vector.memset(epsb[:], EPS)
        A_t = pp.tile([128, NJ * H], F32, tag="At", name="At")
        R_t = pp.tile([128, NJ * H], F32, tag="Rt", name="Rt")
        brows = pp.tile([8, ISL], BF16, tag="brows", name="brows")
        drows = pp.tile([8, ISL], BF16, tag="drows", name="drows")
        Bb = pp.tile([128, H * ISL], BF16, tag="Bb", name="Bb")
        Db = pp.tile([128, H * ISL], BF16, tag="Db", name="Db")
        P2 = pp.tile([128, NJ * ISL], BF16, tag="P2", name="P2")
        Vst = pp.tile([128, VSTASH_H * NJ * ISL], BF16, tag="Vst", name="Vst")
        hcat = [pp.tile([128, ISL], F32, tag=f"hcat{k}", name=f"hcat{k}")
                for k in range(2)]

        # ================= phase 1: projections, e-vectors, broadcasts
        with tc.tile_pool(name="ps1", bufs=1, space="PSUM") as ps1:
            # e_src rows for all heads at once: [8, ISL] (fp32)
            es_ps = ps1.tile([8, ISL], F32, tag="es", name="es")
            for k in range(2):
                nc.tensor.matmul(
                    es_ps[:], wsT[k][:], xTs[k][:],
                    start=(k == 0), stop=(k == 1),
                )
            # scalar queue: Exp set first
            nc.scalar.activation(brows[:], es_ps[:], AF.Exp)
            nc.scalar.activation(drows[:], es_ps[:], AF.Exp, scale=ALPHA)

            # e_dst for all (jt, h) into one PSUM tile: [128, NJ*H] (fp32)
            evall_ps = ps1.tile([128, NJ * H], F32, tag="ev", name="ev")
            for jt in range(NJ):
                c = slice(jt * 128, (jt + 1) * 128)
                cj = slice(jt * H, (jt + 1) * H)
                for k in range(2):
                    nc.tensor.matmul(
                        evall_ps[:, cj], xT[k][:, c], wdT[k][:],
                        start=(k == 0), stop=(k == 1),
                    )
            nc.scalar.activation(A_t[:], evall_ps[:], AF.Exp)
            nc.scalar.activation(R_t[:], evall_ps[:], AF.Exp, scale=ALPHA - 1.0)

            # broadcast B/D rows across partitions: bounce through DRAM,
            # then re-read with a stride-0 partition AP
            nc.sync.dma_start(bd_scr[0:H, :], brows[:])
            nc.sync.dma_start(bd_scr[H:2 * H, :], drows[:])
            for h in range(H):
                ch = slice(h * ISL, (h + 1) * ISL)
                nc.sync.dma_start(Bb[:, ch], bd_scr[h:h + 1, :].partition_broadcast(128))
                nc.sync.dma_start(Db[:, ch], bd_scr[H + h:H + h + 1, :].partition_broadcast(128))

            # projections xp = (x @ W) per j-chunk, packed as 33-blocks
            for jt in range(NJ):
                c = slice(jt * 128, (jt + 1) * 128)
                xp_ps = ps1.tile([128, D], F32, tag="xp", name="xp", bufs=3)
                for k in range(2):
                    nc.tensor.matmul(
                        xp_ps[:], xbf[k][:, c], Wbf[k][:],
                        start=(k == 0), stop=(k == 1),
                    )
                dst = (
                    xp_aug[:, jt * 264:(jt + 1) * 264]
                    .rearrange("p (h w) -> p h w", w=33)[:, :, 0:32]
                )
                src = xp_ps[:].rearrange("p (h w) -> p h w", w=32)
                nc.vector.tensor_copy(dst, src)

            # scalar queue: Ln set (batched), then Exp set (batched)
            for q in range(8):
                cq = slice(q * 1024, (q + 1) * 1024)
                nc.scalar.activation(prT_sb[:, cq], prT_sb[:, cq], AF.Ln, bias=epsb[:])
            for q in range(8):
                cq = slice(q * 1024, (q + 1) * 1024)
                nc.scalar.activation(P2[:, cq], prT_sb[:, cq], AF.Exp, scale=beta)
                # adjacency mask: zero out non-edges (GpSimd, in place)
                nc.gpsimd.tensor_tensor(P2[:, cq], P2[:, cq], adTf[:, cq], OP.mult)

        # ================= phase 2: scores, attention, output
        with tc.tile_pool(name="ps2", bufs=1, space="PSUM") as ps2:
            # v-stash for the first VSTASH_H heads: keeps DVE busy while the
            # scalar engine works through the Ln/Exp chain for P2
            for h in range(VSTASH_H):
                ch = slice(h * ISL, (h + 1) * ISL)
                for jt in range(NJ):
                    ca = jt * H + h
                    sv = slice((h * NJ + jt) * ISL, (h * NJ + jt + 1) * ISL)
                    nc.vector.scalar_tensor_tensor(
                        Vst[:, sv], Db[:, ch], R_t[:, ca:ca + 1], Bb[:, ch],
                        OP.mult, OP.max,
                    )

            if dbg:
                dbg_hts_d = nc.dram_tensor(
                    "dbg_hts", [H * 32, ISL], F32, kind="ExternalOutput").ap()
                dbg_zbt_d = nc.dram_tensor(
                    "dbg_zbt", [H * 32, ISL], F32, kind="ExternalOutput").ap()
                dbg_zin_d = nc.dram_tensor(
                    "dbg_zin", [H, ISL], F32, kind="ExternalOutput").ap()
                dbg_zrow_d = nc.dram_tensor(
                    "dbg_zrow", [H, ISL], F32, kind="ExternalOutput").ap()

            def z_norm(h, hT_ps):
                # hT rows 0..31 are sum_j xp*s, row 32 is Z = sum_j s
                zrow = wk.tile([1, ISL], F32, tag="zrow", name="zrow", bufs=4)
                nc.scalar.copy(zrow[:], hT_ps[32:33, :])
                zin = wk.tile([1, ISL], F32, tag="zin", name="zin", bufs=4)
                nc.vector.reciprocal_approx_fast(zin[:], zrow[:])
                nc.sync.dma_start(z_scr[h:h + 1, :], zin[:])
                zbt = wk.tile([32, ISL], F32, tag="zbt", name="zbt", bufs=2)
                nc.sync.dma_start(zbt[:], z_scr[h:h + 1, :].partition_broadcast(32))
                hts = wk.tile([32, ISL], F32, tag="hts", name="hts", bufs=2)
                nc.scalar.copy(hts[:], hT_ps[0:32, :])
                if dbg:
                    nc.sync.dma_start(dbg_hts_d[h * 32:(h + 1) * 32, :], hts[:])
                    nc.sync.dma_start(dbg_zbt_d[h * 32:(h + 1) * 32, :], zbt[:])
                    nc.sync.dma_start(dbg_zin_d[h:h + 1, :], zin[:])
                    zrow_s = wk.tile([1, ISL], F32, tag="zrow_s", name="zrow_s", bufs=2)
                    nc.scalar.copy(zrow_s[:], hT_ps[32:33, :])
                    nc.sync.dma_start(dbg_zrow_d[h:h + 1, :], zrow_s[:])
                ph = slice((h % 4) * 32, (h % 4) * 32 + 32)
                nc.gpsimd.tensor_tensor(
                    hcat[h // 4][ph, :], hts[:], zbt[:], OP.mult
                )

            for h in range(H):
                ch = slice(h * ISL, (h + 1) * ISL)
                hT_ps = ps2.tile([33, ISL], F32, tag="hT", name="hT", bufs=3)
                for jt in range(NJ):
                    ca = jt * H + h
                    ci = slice(jt * ISL, (jt + 1) * ISL)
                    if h < VSTASH_H:
                        sv = slice((h * NJ + jt) * ISL, (h * NJ + jt + 1) * ISL)
                        v_ap = Vst[:, sv]
                    else:
                        v = wk.tile([128, ISL], BF16, tag="v", name="v", bufs=6)
                        nc.vector.scalar_tensor_tensor(
                            v[:], Db[:, ch], R_t[:, ca:ca + 1], Bb[:, ch],
                            OP.mult, OP.max,
                        )
                        v_ap = v[:]
                    s = wk.tile([128, ISL], BF16, tag="s", name="s", bufs=6)
                    nc.vector.scalar_tensor_tensor(
                        s[:], v_ap, A_t[:, ca:ca + 1], P2[:, ci],
                        OP.mult, OP.mult,
                    )
                    lw = slice(jt * 264 + h * 33, jt * 264 + (h + 1) * 33)
                    nc.tensor.matmul(
                        hT_ps[:], xp_aug[:, lw], s[:],
                        start=(jt == 0), stop=(jt == NJ - 1),
                    )
                z_norm(h, hT_ps)

            if dbg:
                def dump(nm, t, shape, dt):
                    d = nc.dram_tensor(nm, shape, dt, kind="ExternalOutput").ap()
                    nc.sync.dma_start(d, t)
                dump("dbg_Bb", Bb[:], [128, H * ISL], BF16)
                dump("dbg_Db", Db[:], [128, H * ISL], BF16)
                dump("dbg_At", A_t[:], [128, NJ * H], F32)
                dump("dbg_Rt", R_t[:], [128, NJ * H], F32)
                dump("dbg_P2", P2[:], [128, NJ * ISL], BF16)
                dump("dbg_lnp", prT_sb[:], [128, NJ * ISL], F32)
                dump("dbg_xpaug", xp_aug[:], [128, NJ * H * 33], BF16)
                dump("dbg_hcat0", hcat[0][:], [128, ISL], F32)
                dump("dbg_hcat1", hcat[1][:], [128, ISL], F32)
                dump("dbg_Vst", Vst[:], [128, VSTASH_H * NJ * ISL], BF16)

            for ic in range(4):
                cc = slice(ic * 128, (ic + 1) * 128)
                op_ps = ps2.tile([128, D], F32, tag="op", name="op", bufs=2)
                for k in range(2):
                    nc.tensor.matmul(
                        op_ps[:], hcat[k][:, cc], WoT[k][:],
                        start=(k == 0), stop=(k == 1),
                    )
                ob = wk.tile([128, D], F32, tag="ob", name="ob", bufs=2)
                nc.scalar.copy(ob[:], op_ps[:])
                nc.sync.dma_start(out_d[cc, :], ob[:])

    nc.compile()
    return nc


def _get_program(beta: float):
    key = round(beta, 9)
    if key not in _cache:
        _cache[key] = _build(beta)
    return _cache[key]


def kernel(x, adj, prior, W, a_src, a_dst, beta_tilde, W_out, **kw):
    global last_run_info
    x = np.asarray(x, np.float32)
    adj = np.asarray(adj)
    prior = np.asarray(prior, np.float32)
    W = np.asarray(W, np.float32)
    a_src = np.asarray(a_src, np.float32)
    a_dst = np.asarray(a_dst, np.float32)
    W_out = np.asarray(W_out, np.float32)
    assert x.shape == (B, N, D) and prior.shape == (B, N, N)

    bt = float(np.asarray(beta_tilde))
    beta = float(math.log1p(math.exp(bt)))

    nc = _get_program(beta)

    bf16 = mybir.dt.np(BF16)
    xT = np.ascontiguousarray(x.transpose(0, 2, 1))               # [B, D, N]
    Wall = np.ascontiguousarray(W.transpose(1, 0, 2).reshape(D, D))
    WoT = np.ascontiguousarray(W_out.T)
    wsT = np.ascontiguousarray(np.einsum("hdf,hf->hd", W, a_src).T)
    wdT = np.ascontiguousarray(np.einsum("hdf,hf->hd", W, a_dst).T)
    adjT = adj.astype(np.float32).T                               # [j, i]

    in_maps = []
    for c in range(NC):
        b, q = c // 4, c % 4
        i0 = q * ISL
        in_maps.append({
            "xT": xT[b],
            "xTs": np.ascontiguousarray(xT[b][:, i0:i0 + ISL]),
            "prT": np.ascontiguousarray(prior[b, i0:i0 + ISL, :].T),
            "adT": np.ascontiguousarray(adjT[:, i0:i0 + ISL]).astype(bf16),
            "Wall": Wall,
            "WoT": WoT,
            "wsT": wsT,
            "wdT": wdT,
        })

    trace = bool(kw.get("trace", False))
    res = run_bass_kernel_spmd(
        nc, in_maps, core_ids=list(range(NC)), trace=trace
    )
    last_run_info = {
        "exec_time_ns": res.exec_time_ns,
        "mean_exec_time_ns": res.mean_exec_time_ns,
        "trace": res.instructions_and_trace[1]
        if res.instructions_and_trace else None,
    }

    out = np.empty((B, N, D), np.float32)
    for c in range(NC):
        b, q = c // 4, c % 4
        out[b, q * ISL:(q + 1) * ISL, :] = res.results[c]["out"]
    return out


# revision 14
# speedup vs baseline: 1.1627x; 1.1627x over previous
"""Multi-head graph attention kernel for Trainium2, SPMD over 8 NeuronCores.

Sharding: core c owns batch b=c//4 and destination-row slice
i in [512*(c%4), 512*(c%4+1)), all 8 heads.  Each core computes complete
softmax rows (j ranges over all 2048 sources), so there are no cross-core
collectives; the host concatenates the per-core [512, 256] output slabs.

Score pipeline (per core, layout [j=partition, i=free]):
  exp(lrelu(e_src_i + e_dst_j)) = max(A_j*B_i, C_j*D_i)
     A=exp(e_dst), B=exp(e_src), C=exp(alpha*e_dst), D=exp(alpha*e_src)
  Softmax over j is invariant to any per-i scale, so divide by D_i:
     s' = max(A_j*u_i, C_j) * P2,   u_i = B_i/D_i = exp((1-alpha)*e_src_i)
  P2_ji = exp(beta*ln(prior_ji + eps)) * adj_ji    (mask as multiply)
  Per (h, jt) tile this costs just two DVE ops, both in fast perf modes:
     m1 = (Ub * A_j) max C_j      [tensor_scalar, two per-partition scalars]
     s' = m1 * P2                 [tensor_tensor]
  hT[f,i] = sum_j xp[j,f] * s'_ji  (PE, ones-column gives Z_i = sum_j s'_ji)
  out[i,:] = (hT[:,i]/Z) @ W_out.T  (PE, hT layout feeds lhsT directly)

Engine assignment: DVE does the two score ops (+casts, xp copies, adj-mask
multiplies, reciprocals); scalar does all activations (batched by table
set: one Exp group, one Ln group, one Exp group) and PSUM->SBUF copies;
GpSimd does only the Z-normalize multiply; the u-row is broadcast across
partitions via a DRAM bounce + stride-0 DMA.
"""

import math
import sys
from contextlib import ExitStack

sys.path.insert(0, "/opt/trn_rl_repo")

import numpy as np

import concourse.bass as bass
import concourse.tile as tile
from concourse import bacc, mybir
from concourse.bass_utils import run_bass_kernel_spmd

B, N, D, H = 2, 2048, 256, 8
DH = D // H          # 32
NC = 8
ISL = N // 4         # 512 destination rows per core
NJ = N // 128        # 16 j-tiles
EPS = 1e-6
ALPHA = 0.2
MST_H = 2            # heads whose m1-tiles are precomputed into a stash

F32 = mybir.dt.float32
BF16 = mybir.dt.bfloat16

AF = mybir.ActivationFunctionType
OP = mybir.AluOpType

_cache = {}
last_run_info = {}


def _build(beta: float, dbg: bool = False):
    nc = bacc.Bacc(
        "TRN2",
        target_bir_lowering=False,
        debug=False,
        enable_asserts=False,
        num_devices=NC,
    )

    def inp(name, shape, dt):
        return nc.dram_tensor(name, shape, dt, kind="ExternalInput").ap()

    xT_d = inp("xT", [D, N], F32)          # x[b].T
    xTs_d = inp("xTs", [D, ISL], F32)      # x[b, i_slice].T
    prT_d = inp("prT", [N, ISL], F32)      # prior[b, i_slice, :].T
    adT_d = inp("adT", [N, ISL], BF16)     # adj[i_slice, :].T as 0.0/1.0
    Wall_d = inp("Wall", [D, D], F32)      # W head-major columns
    WoT_d = inp("WoT", [D, D], F32)        # W_out.T
    wsT_d = inp("wsT", [D, H], F32)        # (W@a_src per head).T
    wdT_d = inp("wdT", [D, H], F32)        # (W@a_dst per head).T
    out_d = nc.dram_tensor("out", [ISL, D], F32, kind="ExternalOutput").ap()
    # DRAM bounce buffers for partition-broadcasts (SBUF APs cannot have
    # stride-0 partition dims, DRAM APs can)
    u_scr = nc.dram_tensor("uscr", [H, ISL], BF16, kind="Internal").ap()
    z_scr = nc.dram_tensor("zscr", [H, ISL], F32, kind="Internal").ap()

    with tile.TileContext(nc) as tc, ExitStack() as ctx:
        pp = ctx.enter_context(tc.tile_pool(name="persist", bufs=1))
        wk = ctx.enter_context(tc.tile_pool(name="work", bufs=3))

        # ---- resident inputs
        xT = [pp.tile([128, N], F32, tag=f"xT{k}", name=f"xT{k}") for k in range(2)]
        xTs = [pp.tile([128, ISL], F32, tag=f"xTs{k}", name=f"xTs{k}") for k in range(2)]
        Wall = [pp.tile([128, D], F32, tag=f"Wall{k}", name=f"Wall{k}") for k in range(2)]
        WoT = [pp.tile([128, D], F32, tag=f"WoT{k}", name=f"WoT{k}") for k in range(2)]
        wsT = [pp.tile([128, H], F32, tag=f"wsT{k}", name=f"wsT{k}") for k in range(2)]
        wdT = [pp.tile([128, H], F32, tag=f"wdT{k}", name=f"wdT{k}") for k in range(2)]
        for k in range(2):
            r = slice(k * 128, (k + 1) * 128)
            nc.sync.dma_start(xT[k][:], xT_d[r, :])
            nc.sync.dma_start(xTs[k][:], xTs_d[r, :])
            nc.sync.dma_start(Wall[k][:], Wall_d[r, :])
            nc.sync.dma_start(WoT[k][:], WoT_d[r, :])
            nc.sync.dma_start(wsT[k][:], wsT_d[r, :])
            nc.sync.dma_start(wdT[k][:], wdT_d[r, :])

        prT_sb = pp.tile([128, NJ * ISL], F32, tag="prT", name="prT")
        adTf = pp.tile([128, NJ * ISL], BF16, tag="adTf", name="adTf")
        for jt in range(NJ):
            r = slice(jt * 128, (jt + 1) * 128)
            ci = slice(jt * ISL, (jt + 1) * ISL)
            nc.sync.dma_start(prT_sb[:, ci], prT_d[r, :])
            nc.sync.dma_start(adTf[:, ci], adT_d[r, :])

        # bf16 copies of x.T / W for the projection matmul (fp32 matmuls
        # run as HI/LO pairs on the PE - 2x the instructions and cycles)
        xbf = [pp.tile([128, N], BF16, tag=f"xbf{k}", name=f"xbf{k}")
               for k in range(2)]
        Wbf = [pp.tile([128, D], BF16, tag=f"Wbf{k}", name=f"Wbf{k}")
               for k in range(2)]
        for k in range(2):
            nc.vector.tensor_copy(xbf[k][:], xT[k][:])
            nc.vector.tensor_copy(Wbf[k][:], Wall[k][:])

        # ---- persistent intermediates
        xp_aug = pp.tile([128, NJ * H * 33], BF16, tag="xpaug", name="xpaug")
        nc.vector.memset(xp_aug[:], 1.0)  # ones col per 33-block survives
        epsb = pp.tile([128, 1], F32, tag="epsb", name="epsb")
        nc.vector.memset(epsb[:], EPS)
        A_t = pp.tile([128, NJ * H], F32, tag="At", name="At")
        C_t = pp.tile([128, NJ * H], F32, tag="Ct", name="Ct")
        urow = pp.tile([8, ISL], BF16, tag="urow", name="urow")
        Ub = pp.tile([128, H * ISL], BF16, tag="Ub", name="Ub")
        P2 = pp.tile([128, NJ * ISL], BF16, tag="P2", name="P2")
        Mst = pp.tile([128, MST_H * NJ * ISL], BF16, tag="Mst", name="Mst")
        hcat = [pp.tile([128, ISL], F32, tag=f"hcat{k}", name=f"hcat{k}")
                for k in range(2)]

        # ================= phase 1: projections, e-vectors, broadcasts
        with tc.tile_pool(name="ps1", bufs=1, space="PSUM") as ps1:
            # e_src rows for all heads at once: [8, ISL] (fp32)
            es_ps = ps1.tile([8, ISL], F32, tag="es", name="es")
            for k in range(2):
                nc.tensor.matmul(
                    es_ps[:], wsT[k][:], xTs[k][:],
                    start=(k == 0), stop=(k == 1),
                )
            # scalar queue: Exp set first
            nc.scalar.activation(urow[:], es_ps[:], AF.Exp, scale=1.0 - ALPHA)

            # e_dst for all (jt, h) into one PSUM tile: [128, NJ*H] (fp32)
            evall_ps = ps1.tile([128, NJ * H], F32, tag="ev", name="ev")
            for jt in range(NJ):
                c = slice(jt * 128, (jt + 1) * 128)
                cj = slice(jt * H, (jt + 1) * H)
                for k in range(2):
                    nc.tensor.matmul(
                        evall_ps[:, cj], xT[k][:, c], wdT[k][:],
                        start=(k == 0), stop=(k == 1),
                    )
            nc.scalar.activation(A_t[:], evall_ps[:], AF.Exp)
            nc.scalar.activation(C_t[:], evall_ps[:], AF.Exp, scale=ALPHA)

            # broadcast the u-row across partitions: bounce through DRAM,
            # then re-read with a stride-0 partition AP
            nc.sync.dma_start(u_scr[0:H, :], urow[:])
            for h in range(H):
                ch = slice(h * ISL, (h + 1) * ISL)
                nc.sync.dma_start(
                    Ub[:, ch], u_scr[h:h + 1, :].partition_broadcast(128))

            # projections xp = (x @ W) per j-chunk, packed as 33-blocks
            for jt in range(NJ):
                c = slice(jt * 128, (jt + 1) * 128)
                xp_ps = ps1.tile([128, D], F32, tag="xp", name="xp", bufs=3)
                for k in range(2):
                    nc.tensor.matmul(
                        xp_ps[:], xbf[k][:, c], Wbf[k][:],
                        start=(k == 0), stop=(k == 1),
                    )
                dst = (
                    xp_aug[:, jt * 264:(jt + 1) * 264]
                    .rearrange("p (h w) -> p h w", w=33)[:, :, 0:32]
                )
                src = xp_ps[:].rearrange("p (h w) -> p h w", w=32)
                nc.vector.tensor_copy(dst, src)

            # scalar queue: Ln set (batched), then Exp set (batched)
            for q in range(8):
                cq = slice(q * 1024, (q + 1) * 1024)
                nc.scalar.activation(prT_sb[:, cq], prT_sb[:, cq], AF.Ln,
                                     bias=epsb[:])
            for q in range(8):
                cq = slice(q * 1024, (q + 1) * 1024)
                nc.scalar.activation(P2[:, cq], prT_sb[:, cq], AF.Exp,
                                     scale=beta)

        # ================= phase 2: scores, attention, output
        with tc.tile_pool(name="ps2", bufs=1, space="PSUM") as ps2:
            # m1-stash for the first MST_H heads, interleaved with the
            # adj-mask multiplies: keeps DVE busy while the scalar engine
            # works through the Ln/Exp chain for P2, and applies the mask
            # as soon as each Exp chunk lands
            mq = 0
            for h in range(MST_H):
                ch = slice(h * ISL, (h + 1) * ISL)
                for jt in range(NJ):
                    ca = jt * H + h
                    sv = slice((h * NJ + jt) * ISL, (h * NJ + jt + 1) * ISL)
                    nc.vector.tensor_scalar(
                        Mst[:, sv], Ub[:, ch],
                        A_t[:, ca:ca + 1], C_t[:, ca:ca + 1],
                        OP.mult, OP.max,
                    )
                    if (h * NJ + jt) % 4 == 3 and mq < 8:
                        cq = slice(mq * 1024, (mq + 1) * 1024)
                        nc.vector.tensor_tensor(
                            P2[:, cq], P2[:, cq], adTf[:, cq], OP.mult)
                        mq += 1

            if dbg:
                dbg_hts_d = nc.dram_tensor(
                    "dbg_hts", [H * 32, ISL], F32, kind="ExternalOutput").ap()
                dbg_zbt_d = nc.dram_tensor(
                    "dbg_zbt", [H * 32, ISL], F32, kind="ExternalOutput").ap()

            def z_norm(h, hT_ps):
                # hT rows 0..31 are sum_j xp*s, row 32 is Z = sum_j s
                zrow = wk.tile([1, ISL], F32, tag="zrow", name="zrow", bufs=4)
                nc.scalar.copy(zrow[:], hT_ps[32:33, :])
                zin = wk.tile([1, ISL], F32, tag="zin", name="zin", bufs=4)
                nc.vector.reciprocal_approx_fast(zin[:], zrow[:])
                nc.sync.dma_start(z_scr[h:h + 1, :], zin[:])
                zbt = wk.tile([32, ISL], F32, tag="zbt", name="zbt", bufs=2)
                nc.sync.dma_start(zbt[:], z_scr[h:h + 1, :].partition_broadcast(32))
                hts = wk.tile([32, ISL], F32, tag="hts", name="hts", bufs=2)
                nc.scalar.copy(hts[:], hT_ps[0:32, :])
                if dbg:
                    nc.sync.dma_start(dbg_hts_d[h * 32:(h + 1) * 32, :], hts[:])
                    nc.sync.dma_start(dbg_zbt_d[h * 32:(h + 1) * 32, :], zbt[:])
                ph = slice((h % 4) * 32, (h % 4) * 32 + 32)
                nc.gpsimd.tensor_tensor(
                    hcat[h // 4][ph, :], hts[:], zbt[:], OP.mult
                )

            for h in range(H):
                ch = slice(h * ISL, (h + 1) * ISL)
                hT_ps = ps2.tile([33, ISL], F32, tag="hT", name="hT", bufs=3)
                for jt in range(NJ):
                    ca = jt * H + h
                    ci = slice(jt * ISL, (jt + 1) * ISL)
                    if h < MST_H:
                        sv = slice((h * NJ + jt) * ISL, (h * NJ + jt + 1) * ISL)
                        m_ap = Mst[:, sv]
                    else:
                        m1 = wk.tile([128, ISL], BF16, tag="m1", name="m1", bufs=6)
                        nc.vector.tensor_scalar(
                            m1[:], Ub[:, ch],
                            A_t[:, ca:ca + 1], C_t[:, ca:ca + 1],
                            OP.mult, OP.max,
                        )
                        m_ap = m1[:]
                    s = wk.tile([128, ISL], BF16, tag="s", name="s", bufs=6)
                    nc.vector.tensor_tensor(s[:], m_ap, P2[:, ci], OP.mult)
                    lw = slice(jt * 264 + h * 33, jt * 264 + (h + 1) * 33)
                    nc.tensor.matmul(
                        hT_ps[:], xp_aug[:, lw], s[:],
                        start=(jt == 0), stop=(jt == NJ - 1),
                    )
                z_norm(h, hT_ps)

            if dbg:
                def dump(nm, t, shape, dt):
                    d = nc.dram_tensor(nm, shape, dt, kind="ExternalOutput").ap()
                    nc.sync.dma_start(d, t)
                dump("dbg_Ub", Ub[:], [128, H * ISL], BF16)
                dump("dbg_At", A_t[:], [128, NJ * H], F32)
                dump("dbg_Ct", C_t[:], [128, NJ * H], F32)
                dump("dbg_P2", P2[:], [128, NJ * ISL], BF16)
                dump("dbg_lnp", prT_sb[:], [128, NJ * ISL], F32)
                dump("dbg_xpaug", xp_aug[:], [128, NJ * H * 33], BF16)
                dump("dbg_hcat0", hcat[0][:], [128, ISL], F32)
                dump("dbg_hcat1", hcat[1][:], [128, ISL], F32)
                dump("dbg_Mst", Mst[:], [128, MST_H * NJ * ISL], BF16)

            for ic in range(4):
                cc = slice(ic * 128, (ic + 1) * 128)
                op_ps = ps2.tile([128, D], F32, tag="op", name="op", bufs=2)
                for k in range(2):
                    nc.tensor.matmul(
                        op_ps[:], hcat[k][:, cc], WoT[k][:],
                        start=(k == 0), stop=(k == 1),
                    )
                ob = wk.tile([128, D], F32, tag="ob", name="ob", bufs=2)
                nc.scalar.copy(ob[:], op_ps[:])
                nc.sync.dma_start(out_d[cc, :], ob[:])

    nc.compile()
    return nc


def _get_program(beta: float):
    key = round(beta, 9)
    if key not in _cache:
        _cache[key] = _build(beta)
    return _cache[key]


def kernel(x, adj, prior, W, a_src, a_dst, beta_tilde, W_out, **kw):
    global last_run_info
    x = np.asarray(x, np.float32)
    adj = np.asarray(adj)
    prior = np.asarray(prior, np.float32)
    W = np.asarray(W, np.float32)
    a_src = np.asarray(a_src, np.float32)
    a_dst = np.asarray(a_dst, np.float32)
    W_out = np.asarray(W_out, np.float32)
    assert x.shape == (B, N, D) and prior.shape == (B, N, N)

    bt = float(np.asarray(beta_tilde))
    beta = float(math.log1p(math.exp(bt)))

    nc = _get_program(beta)

    bf16 = mybir.dt.np(BF16)
    xT = np.ascontiguousarray(x.transpose(0, 2, 1))               # [B, D, N]
    Wall = np.ascontiguousarray(W.transpose(1, 0, 2).reshape(D, D))
    WoT = np.ascontiguousarray(W_out.T)
    wsT = np.ascontiguousarray(np.einsum("hdf,hf->hd", W, a_src).T)
    wdT = np.ascontiguousarray(np.einsum("hdf,hf->hd", W, a_dst).T)
    adjT = adj.astype(np.float32).T                               # [j, i]

    in_maps = []
    for c in range(NC):
        b, q = c // 4, c % 4
        i0 = q * ISL
        in_maps.append({
            "xT": xT[b],
            "xTs": np.ascontiguousarray(xT[b][:, i0:i0 + ISL]),
            "prT": np.ascontiguousarray(prior[b, i0:i0 + ISL, :].T),
            "adT": np.ascontiguousarray(adjT[:, i0:i0 + ISL]).astype(bf16),
            "Wall": Wall,
            "WoT": WoT,
            "wsT": wsT,
            "wdT": wdT,
        })

    trace = bool(kw.get("trace", False))
    res = run_bass_kernel_spmd(
        nc, in_maps, core_ids=list(range(NC)), trace=trace
    )
    last_run_info = {
        "exec_time_ns": res.exec_time_ns,
        "mean_exec_time_ns": res.mean_exec_time_ns,
        "trace": res.instructions_and_trace[1]
        if res.instructions_and_trace else None,
    }

    out = np.empty((B, N, D), np.float32)
    for c in range(NC):
        b, q = c // 4, c % 4
        out[b, q * ISL:(q + 1) * ISL, :] = res.results[c]["out"]
    return out


# revision 16
# speedup vs baseline: 1.6606x; 1.4282x over previous
"""Multi-head graph attention kernel for Trainium2, SPMD over 8 NeuronCores.

Sharding: core c owns batch b=c//4 and destination-row slice
i in [512*(c%4), 512*(c%4+1)), all 8 heads.  Each core computes complete
softmax rows (j ranges over all 2048 sources), so there are no cross-core
collectives; the host concatenates the per-core [512, 256] output slabs.

Score pipeline (per core, layout [j=partition, i=free]):
  exp(lrelu(e_src_i + e_dst_j)) = max(A_j*B_i, C_j*D_i)
     A=exp(e_dst), B=exp(e_src), C=exp(alpha*e_dst), D=exp(alpha*e_src)
  Softmax over j is invariant to any per-i scale, so divide by D_i:
     s' = max(A_j*u_i, C_j) * P2,   u_i = B_i/D_i = exp((1-alpha)*e_src_i)
  P2_ji = exp(beta*ln(prior_ji + eps)) * adj_ji    (mask as multiply)
  Per (h, jt) tile: one tensor_scalar (two per-partition scalars) builds
  max(A*u, C); heads are processed in pairs so a single [128, 1024]
  tensor_tensor applies the head-independent P2 (free-dim-broadcast AP)
  to both heads at once.
  hT[f,i] = sum_j xp[j,f] * s'_ji  (PE, ones-column gives Z_i = sum_j s'_ji)
  out[i,:] = (hT[:,i]/Z) @ W_out.T  (PE, hT layout feeds lhsT directly)

Engine assignment: DVE does the score ops, xp copies, adj-mask multiplies
and reciprocals; scalar does all activations and PSUM->SBUF copies -- with
explicit dependency chains (epsb/betas8) forcing all-Exp, all-Ln, all-Exp
order so only 3 activation-table loads happen; GpSimd does only the
Z-normalize multiply; u-row and 1/Z rows are broadcast across partitions
via a DRAM bounce + stride-0 DMA.
"""

import math
import sys
from contextlib import ExitStack

sys.path.insert(0, "/opt/trn_rl_repo")

import numpy as np

import concourse.bass as bass
import concourse.tile as tile
from concourse import bacc, mybir
from concourse.bass_utils import run_bass_kernel_spmd

B, N, D, H = 2, 2048, 256, 8
DH = D // H          # 32
NC = 8
ISL = N // 4         # 512 destination rows per core
NJ = N // 128        # 16 j-tiles
EPS = 1e-6
ALPHA = 0.2

F32 = mybir.dt.float32
BF16 = mybir.dt.bfloat16

AF = mybir.ActivationFunctionType
OP = mybir.AluOpType

_cache = {}
last_run_info = {}


def _build(beta: float, dbg: bool = False):
    nc = bacc.Bacc(
        "TRN2",
        target_bir_lowering=False,
        debug=False,
        enable_asserts=False,
        num_devices=NC,
    )

    def inp(name, shape, dt):
        return nc.dram_tensor(name, shape, dt, kind="ExternalInput").ap()

    xbf_d = inp("xbf", [D, N], BF16)       # x[b].T cast bf16 (host)
    xTs_d = inp("xTs", [D, ISL], F32)      # x[b, i_slice].T
    prT_d = inp("prT", [N, ISL], F32)      # prior[b, i_slice, :].T
    adT_d = inp("adT", [N, ISL], BF16)     # adj[i_slice, :].T as 0.0/1.0
    Wbf_d = inp("Wbf", [D, D], BF16)       # W head-major columns, bf16
    WoT_d = inp("WoT", [D, D], F32)        # W_out.T
    wsT_d = inp("wsT", [D, H], F32)        # (W@a_src per head).T
    wdbf_d = inp("wdbf", [D, H], BF16)     # (W@a_dst per head).T, bf16
    out_d = nc.dram_tensor("out", [ISL, D], F32, kind="ExternalOutput").ap()
    # DRAM bounce buffers for partition-broadcasts (SBUF APs cannot have
    # stride-0 partition dims, DRAM APs can)
    u_scr = nc.dram_tensor("uscr", [H, ISL], BF16, kind="Internal").ap()
    z_scr = nc.dram_tensor("zscr", [H, ISL], F32, kind="Internal").ap()

    with tile.TileContext(nc) as tc, ExitStack() as ctx:
        pp = ctx.enter_context(tc.tile_pool(name="persist", bufs=1))
        wk = ctx.enter_context(tc.tile_pool(name="work", bufs=3))

        # ---- resident inputs
        xbf = [pp.tile([128, N], BF16, tag=f"xbf{k}", name=f"xbf{k}") for k in range(2)]
        xTs = [pp.tile([128, ISL], F32, tag=f"xTs{k}", name=f"xTs{k}") for k in range(2)]
        Wbf = [pp.tile([128, D], BF16, tag=f"Wbf{k}", name=f"Wbf{k}") for k in range(2)]
        WoT = [pp.tile([128, D], F32, tag=f"WoT{k}", name=f"WoT{k}") for k in range(2)]
        wsT = [pp.tile([128, H], F32, tag=f"wsT{k}", name=f"wsT{k}") for k in range(2)]
        wdbf = [pp.tile([128, H], BF16, tag=f"wdbf{k}", name=f"wdbf{k}") for k in range(2)]
        for k in range(2):
            r = slice(k * 128, (k + 1) * 128)
            nc.sync.dma_start(xbf[k][:], xbf_d[r, :])
            nc.sync.dma_start(xTs[k][:], xTs_d[r, :])
            nc.sync.dma_start(Wbf[k][:], Wbf_d[r, :])
            nc.sync.dma_start(WoT[k][:], WoT_d[r, :])
            nc.sync.dma_start(wsT[k][:], wsT_d[r, :])
            nc.sync.dma_start(wdbf[k][:], wdbf_d[r, :])

        prT_sb = pp.tile([128, NJ * ISL], F32, tag="prT", name="prT")
        for jt in range(NJ):
            r = slice(jt * 128, (jt + 1) * 128)
            ci = slice(jt * ISL, (jt + 1) * ISL)
            nc.sync.dma_start(prT_sb[:, ci], prT_d[r, :])

        # ---- persistent intermediates
        xp_aug = pp.tile([128, NJ * H * 33], BF16, tag="xpaug", name="xpaug")
        nc.vector.memset(xp_aug[:], 1.0)  # ones col per 33-block survives
        A_t = pp.tile([128, NJ * H], F32, tag="At", name="At")
        C_t = pp.tile([128, NJ * H], F32, tag="Ct", name="Ct")
        urow = pp.tile([8, ISL], BF16, tag="urow", name="urow")
        Ub = pp.tile([128, H * ISL], BF16, tag="Ub", name="Ub")
        P2 = pp.tile([128, NJ * ISL], BF16, tag="P2", name="P2")
        # eps-bias tiles, written on the scalar engine AFTER the phase-1
        # exps so the scheduler cannot interleave Ln into the Exp group
        epsb = pp.tile([128, 1], F32, tag="epsb", name="epsb")
        epsb2 = pp.tile([128, 1], F32, tag="epsb2", name="epsb2")
        betas8 = pp.tile([128, 8], F32, tag="betas8", name="betas8")
        # m1-stash for head pair 0: [jt, (h0 | h1)] layout
        Mst = pp.tile([128, NJ * 2 * ISL], BF16, tag="Mst", name="Mst")
        hcat = [pp.tile([128, ISL], F32, tag=f"hcat{k}", name=f"hcat{k}")
                for k in range(2)]

        # ================= phase 1: projections, e-vectors, broadcasts
        with tc.tile_pool(name="ps1", bufs=1, space="PSUM") as ps1:
            # e_src rows for all heads at once: [8, ISL] (fp32)
            es_ps = ps1.tile([8, ISL], F32, tag="es", name="es")
            for k in range(2):
                nc.tensor.matmul(
                    es_ps[:], wsT[k][:], xTs[k][:],
                    start=(k == 0), stop=(k == 1),
                )
            # scalar queue: Exp set first
            nc.scalar.activation(urow[:], es_ps[:], AF.Exp, scale=1.0 - ALPHA)

            # e_dst for all (jt, h) into one PSUM tile: [128, NJ*H] (bf16 in)
            evall_ps = ps1.tile([128, NJ * H], F32, tag="ev", name="ev")
            for jt in range(NJ):
                c = slice(jt * 128, (jt + 1) * 128)
                cj = slice(jt * H, (jt + 1) * H)
                for k in range(2):
                    nc.tensor.matmul(
                        evall_ps[:, cj], xbf[k][:, c], wdbf[k][:],
                        start=(k == 0), stop=(k == 1),
                    )
            nc.scalar.activation(A_t[:], evall_ps[:], AF.Exp)
            nc.scalar.activation(C_t[:], evall_ps[:], AF.Exp, scale=ALPHA)
            # eps tiles: Copy with scale=0 -> constant EPS, but data-dependent
            # on A_t/C_t so every Ln schedules after the phase-1 Exps
            nc.scalar.activation(epsb[:], A_t[:, 0:1], AF.Copy,
                                 bias=EPS, scale=0.0)
            nc.scalar.activation(epsb2[:], C_t[:, 0:1], AF.Copy,
                                 bias=EPS, scale=0.0)

            # broadcast the u-row across partitions: bounce through DRAM,
            # then re-read with a stride-0 partition AP
            nc.sync.dma_start(u_scr[0:H, :], urow[:])
            for h in range(H):
                ch = slice(h * ISL, (h + 1) * ISL)
                nc.sync.dma_start(
                    Ub[:, ch], u_scr[h:h + 1, :].partition_broadcast(128))

            # projections xp = (x @ W) per j-chunk, packed as 33-blocks
            for jt in range(NJ):
                c = slice(jt * 128, (jt + 1) * 128)
                xp_ps = ps1.tile([128, D], F32, tag="xp", name="xp", bufs=3)
                for k in range(2):
                    nc.tensor.matmul(
                        xp_ps[:], xbf[k][:, c], Wbf[k][:],
                        start=(k == 0), stop=(k == 1),
                    )
                dst = (
                    xp_aug[:, jt * 264:(jt + 1) * 264]
                    .rearrange("p (h w) -> p h w", w=33)[:, :, 0:32]
                )
                src = xp_ps[:].rearrange("p (h w) -> p h w", w=32)
                nc.vector.tensor_copy(dst, src)

            # scalar queue: Ln set (batched; first two chunks carry the
            # dependency on the phase-1 Exps via the eps bias tiles)
            for q in range(8):
                cq = slice(q * 1024, (q + 1) * 1024)
                bias = epsb2 if q == 1 else epsb
                nc.scalar.activation(prT_sb[:, cq], prT_sb[:, cq], AF.Ln,
                                     bias=bias[:])
            # beta-scale tile: strided read touches every Ln chunk, so all
            # Exps schedule after all Lns (3 table loads total)
            lncols = (prT_sb[:].rearrange("p (q w) -> p q w", w=1024)
                      [:, :, 1023:1024])
            nc.scalar.activation(betas8[:], lncols, AF.Copy,
                                 bias=beta, scale=0.0)
            for q in range(8):
                cq = slice(q * 1024, (q + 1) * 1024)
                nc.scalar.activation(P2[:, cq], prT_sb[:, cq], AF.Exp,
                                     scale=betas8[:, 0:1])

        # ================= phase 2: scores, attention, output
        with tc.tile_pool(name="ps2", bufs=1, space="PSUM") as ps2:
            # m1-stash for head pair (0,1), interleaved with the adj-mask
            # multiplies: keeps DVE busy while the scalar engine works
            # through the Ln/Exp chain for P2, and applies the mask as soon
            # as each Exp chunk lands
            adq_tiles = {}
            mq = 0
            for jt in range(NJ):
                for hh in range(2):
                    ca = jt * H + hh
                    ch = slice(hh * ISL, (hh + 1) * ISL)
                    sv = slice(jt * 1024 + hh * ISL, jt * 1024 + (hh + 1) * ISL)
                    nc.vector.tensor_scalar(
                        Mst[:, sv], Ub[:, ch],
                        A_t[:, ca:ca + 1], C_t[:, ca:ca + 1],
                        OP.mult, OP.max,
                    )
                if jt % 2 == 1 and mq < 8:
                    cq = slice(mq * 1024, (mq + 1) * 1024)
                    adq = wk.tile([128, 1024], BF16, tag="adq", name="adq",
                                  bufs=2)
                    for half in range(2):
                        r = slice((2 * mq + half) * 128, (2 * mq + half + 1) * 128)
                        nc.sync.dma_start(
                            adq[:, half * ISL:(half + 1) * ISL], adT_d[r, :])
                    nc.vector.tensor_tensor(
                        P2[:, cq], P2[:, cq], adq[:], OP.mult)
                    mq += 1

            if dbg:
                dbg_hts_d = nc.dram_tensor(
                    "dbg_hts", [H * 32, ISL], F32, kind="ExternalOutput").ap()
                dbg_zbt_d = nc.dram_tensor(
                    "dbg_zbt", [H * 32, ISL], F32, kind="ExternalOutput").ap()

            def z_norm(h, hT_ps):
                # hT rows 0..31 are sum_j xp*s, row 32 is Z = sum_j s
                zrow = wk.tile([1, ISL], F32, tag="zrow", name="zrow", bufs=4)
                nc.scalar.copy(zrow[:], hT_ps[32:33, :])
                zin = wk.tile([1, ISL], F32, tag="zin", name="zin", bufs=4)
                nc.vector.reciprocal_approx_fast(zin[:], zrow[:])
                nc.sync.dma_start(z_scr[h:h + 1, :], zin[:])
                zbt = wk.tile([32, ISL], F32, tag="zbt", name="zbt", bufs=2)
                nc.sync.dma_start(zbt[:], z_scr[h:h + 1, :].partition_broadcast(32))
                hts = wk.tile([32, ISL], F32, tag="hts", name="hts", bufs=2)
                nc.scalar.copy(hts[:], hT_ps[0:32, :])
                if dbg:
                    nc.sync.dma_start(dbg_hts_d[h * 32:(h + 1) * 32, :], hts[:])
                    nc.sync.dma_start(dbg_zbt_d[h * 32:(h + 1) * 32, :], zbt[:])
                ph = slice((h % 4) * 32, (h % 4) * 32 + 32)
                nc.gpsimd.tensor_tensor(
                    hcat[h // 4][ph, :], hts[:], zbt[:], OP.mult
                )

            for hp in range(4):
                ha, hb = 2 * hp, 2 * hp + 1
                psA = ps2.tile([33, ISL], F32, tag="hT", name="hTa", bufs=4)
                psB = ps2.tile([33, ISL], F32, tag="hT", name="hTb", bufs=4)
                for jt in range(NJ):
                    ci = slice(jt * ISL, (jt + 1) * ISL)
                    if hp == 0:
                        spair = slice(jt * 1024, (jt + 1) * 1024)
                        m_ap = Mst[:, spair]
                    else:
                        m2 = wk.tile([128, 2 * ISL], BF16, tag="m2",
                                     name="m2", bufs=4)
                        for hh, h in ((0, ha), (1, hb)):
                            ca = jt * H + h
                            nc.vector.tensor_scalar(
                                m2[:, hh * ISL:(hh + 1) * ISL],
                                Ub[:, h * ISL:(h + 1) * ISL],
                                A_t[:, ca:ca + 1], C_t[:, ca:ca + 1],
                                OP.mult, OP.max,
                            )
                        m_ap = m2[:]
                    s2 = wk.tile([128, 2 * ISL], BF16, tag="s2", name="s2",
                                 bufs=4)
                    nc.vector.tensor_tensor(
                        s2[:].rearrange("p (two i) -> p two i", two=2),
                        m_ap.rearrange("p (two i) -> p two i", two=2),
                        P2[:, ci][:, None, :].to_broadcast([128, 2, ISL]),
                        OP.mult,
                    )
                    for hh, h in ((0, ha), (1, hb)):
                        lw = slice(jt * 264 + h * 33, jt * 264 + (h + 1) * 33)
                        ps = psA if hh == 0 else psB
                        nc.tensor.matmul(
                            ps[:], xp_aug[:, lw],
                            s2[:, hh * ISL:(hh + 1) * ISL],
                            start=(jt == 0), stop=(jt == NJ - 1),
                        )
                z_norm(ha, psA)
                z_norm(hb, psB)

            if dbg:
                def dump(nm, t, shape, dt):
                    d = nc.dram_tensor(nm, shape, dt, kind="ExternalOutput").ap()
                    nc.sync.dma_start(d, t)
                dump("dbg_Ub", Ub[:], [128, H * ISL], BF16)
                dump("dbg_At", A_t[:], [128, NJ * H], F32)
                dump("dbg_Ct", C_t[:], [128, NJ * H], F32)
                dump("dbg_P2", P2[:], [128, NJ * ISL], BF16)
                dump("dbg_lnp", prT_sb[:], [128, NJ * ISL], F32)
                dump("dbg_xpaug", xp_aug[:], [128, NJ * H * 33], BF16)
                dump("dbg_hcat0", hcat[0][:], [128, ISL], F32)
                dump("dbg_hcat1", hcat[1][:], [128, ISL], F32)
                dump("dbg_Mst", Mst[:], [128, NJ * 2 * ISL], BF16)

            for ic in range(4):
                cc = slice(ic * 128, (ic + 1) * 128)
                op_ps = ps2.tile([128, D], F32, tag="op", name="op", bufs=2)
                for k in range(2):
                    nc.tensor.matmul(
                        op_ps[:], hcat[k][:, cc], WoT[k][:],
                        start=(k == 0), stop=(k == 1),
                    )
                ob = wk.tile([128, D], F32, tag="ob", name="ob", bufs=2)
                nc.scalar.copy(ob[:], op_ps[:])
                nc.sync.dma_start(out_d[cc, :], ob[:])

    nc.compile()
    return nc


def _get_program(beta: float):
    key = round(beta, 9)
    if key not in _cache:
        _cache[key] = _build(beta)
    return _cache[key]


def kernel(x, adj, prior, W, a_src, a_dst, beta_tilde, W_out, **kw):
    global last_run_info
    x = np.asarray(x, np.float32)
    adj = np.asarray(adj)
    prior = np.asarray(prior, np.float32)
    W = np.asarray(W, np.float32)
    a_src = np.asarray(a_src, np.float32)
    a_dst = np.asarray(a_dst, np.float32)
    W_out = np.asarray(W_out, np.float32)
    assert x.shape == (B, N, D) and prior.shape == (B, N, N)

    bt = float(np.asarray(beta_tilde))
    beta = float(math.log1p(math.exp(bt)))

    nc = _get_program(beta)

    bf16 = mybir.dt.np(BF16)
    xT = np.ascontiguousarray(x.transpose(0, 2, 1))               # [B, D, N]
    xbf = xT.astype(bf16)
    Wbf = np.ascontiguousarray(
        W.transpose(1, 0, 2).reshape(D, D)).astype(bf16)
    WoT = np.ascontiguousarray(W_out.T)
    wsT = np.ascontiguousarray(np.einsum("hdf,hf->hd", W, a_src).T)
    wdbf = np.ascontiguousarray(
        np.einsum("hdf,hf->hd", W, a_dst).T).astype(bf16)
    adjT = adj.astype(np.float32).T                               # [j, i]

    in_maps = []
    for c in range(NC):
        b, q = c // 4, c % 4
        i0 = q * ISL
        in_maps.append({
            "xbf": xbf[b],
            "xTs": np.ascontiguousarray(xT[b][:, i0:i0 + ISL]),
            "prT": np.ascontiguousarray(prior[b, i0:i0 + ISL, :].T),
            "adT": np.ascontiguousarray(adjT[:, i0:i0 + ISL]).astype(bf16),
            "Wbf": Wbf,
            "WoT": WoT,
            "wsT": wsT,
            "wdbf": wdbf,
        })

    trace = bool(kw.get("trace", False))
    res = run_bass_kernel_spmd(
        nc, in_maps, core_ids=list(range(NC)), trace=trace
    )
    last_run_info = {
        "exec_time_ns": res.exec_time_ns,
        "mean_exec_time_ns": res.mean_exec_time_ns,
        "trace": res.instructions_and_trace[1]
        if res.instructions_and_trace else None,
    }

    out = np.empty((B, N, D), np.float32)
    for c in range(NC):
        b, q = c // 4, c % 4
        out[b, q * ISL:(q + 1) * ISL, :] = res.results[c]["out"]
    return out
